# revision 15
# baseline (speedup 1.0000x reference)
"""Trainium2 Bass kernel for nn_CTANet (segment_reduce).

Reference math (per batch element b):
  s_proto[c]   = sum_hw sf[b,c,hw] * label[b,hw] / sum(label)      (global sum)
  q_fsum[k,c]  = sum_{hw: seg[b,hw]==k} qf[b,c,hw];  counts[k] = |{hw: seg==k}|
  cos[b,k]     = <s_proto, q_fsum[k]/max(counts,1)> / max(|s_proto|*|q_proto|, eps)
  keep[b,k]    = present & (cos>=0 | k==255 | npres==1);  keepf = 255*keep
  final[b,y,x] = keepf[b, query_mask[b,y,x]]

Strategy: pure data-parallel over batch (8 items per NeuronCore, 8 cores).
All compute per core is self-contained (global label sum is computed
redundantly on each core from the full label tensor -> no collectives).

Per core:
  - s_proto via PE-broadcast of the label row to PSUM + fused DVE
    multiply-reduce against s_feature in natural [c, hw] layout.
  - q_feature transposed to [hw, c] chunks on the PE (identity matmul),
    one-hot [hw, k] built with 2 ScalarE activation passes
    (relu(1 - |iota - seg|)), then PE matmul onehot^T @ [qf_T | ones]
    accumulating over hw chunks -> q_fsum[k, 0:256], counts at col 256.
  - cosine/keep decisions as small [128, b*kc] elementwise DVE/ACT ops.
  - final mask: per-batch PE matmul of the retained one-hot tiles with the
    0/1 keep column -> 28x28 mask, upsampled 16x16 to 448x448 via a dup-AP
    DVE copy (x) and a replicating-source DMA (y).

query_mask is exactly repeat(small_q_mask, 16, 16) for inputs produced by
reference.setup_inputs(); the host verifies this cheaply and falls back to
a host-side gather if it ever does not hold (correctness safety net).
"""

import numpy as np

B, C, HS, K, UP = 64, 256, 28, 256, 16
HW = HS * HS            # 784
HQ = HS * UP            # 448
NCORES = 8
BPC = B // NCORES       # 8 batches per core
HWC = 112               # hw chunk (7 chunks of 112)
NHWC = HW // HWC        # 7
EPS = 1e-8

_CACHE = {}


def _build_nc():
    from contextlib import ExitStack

    import concourse.bass as bass
    import concourse.tile as tile
    from concourse import mybir

    f32 = mybir.dt.float32
    i16 = mybir.dt.int16
    Alu = mybir.AluOpType
    Act = mybir.ActivationFunctionType

    nc = bass.Bass(trn_type="TRN2")

    # ---- I/O ----------------------------------------------------------
    sf_d = nc.dram_tensor("sf", [BPC, 2, 128, HW], f32, kind="ExternalInput")
    qf_d = nc.dram_tensor("qf", [BPC, 2, 128, HW], f32, kind="ExternalInput")
    laball_d = nc.dram_tensor("label_all", [128, B * HW // 128], f32,
                              kind="ExternalInput")
    labown_d = nc.dram_tensor("label_own", [1, BPC * HW], f32, kind="ExternalInput")
    segn_d = nc.dram_tensor("segn", [HWC, BPC * NHWC], f32, kind="ExternalInput")
    iota_d = nc.dram_tensor("iota_row", [128, K], f32, kind="ExternalInput")
    ident_d = nc.dram_tensor("ident", [128, 128], f32, kind="ExternalInput")
    onesr_d = nc.dram_tensor("ones_row", [1, 128], f32, kind="ExternalInput")
    onesc_d = nc.dram_tensor("ones_col", [128, 1], f32, kind="ExternalInput")
    kpos_d = nc.dram_tensor("kpos", [128, 2], f32, kind="ExternalInput")
    kidx_d = nc.dram_tensor("kidx", [128, 2], f32, kind="ExternalInput")
    segrow_d = nc.dram_tensor("segrow", [1, BPC * HW], f32, kind="ExternalInput")
    k255_d = nc.dram_tensor("k255", [128, 2], f32, kind="ExternalInput")

    outm_d = nc.dram_tensor("outm", [BPC, HQ, HQ], f32, kind="ExternalOutput")
    cos_d = nc.dram_tensor("cos_out", [2 * BPC, 128], f32, kind="ExternalOutput")

    with tile.TileContext(nc) as tc, ExitStack() as ctx:
        consts = ctx.enter_context(tc.tile_pool(name="consts", bufs=1))
        feat = ctx.enter_context(tc.tile_pool(name="feat", bufs=2))
        qft_p = ctx.enter_context(tc.tile_pool(name="qft", bufs=9))
        oh_p = ctx.enter_context(tc.tile_pool(name="oh", bufs=9))
        scr_p = ctx.enter_context(tc.tile_pool(name="scr", bufs=3))
        small = ctx.enter_context(tc.tile_pool(name="small", bufs=4))
        keep_p = ctx.enter_context(tc.tile_pool(name="keep", bufs=1))
        g_p = ctx.enter_context(tc.tile_pool(name="g", bufs=2))

        ps_lab = ctx.enter_context(tc.tile_pool(name="pslab", bufs=1, space="PSUM"))
        ps_t = ctx.enter_context(tc.tile_pool(name="pst", bufs=2, space="PSUM"))
        ps_q = ctx.enter_context(tc.tile_pool(name="psq", bufs=4, space="PSUM"))

        # ---- constants ------------------------------------------------
        id_sb = consts.tile([128, 128], f32)
        nc.sync.dma_start(id_sb, ident_d[:, :])
        iota_sb = consts.tile([128, K], f32)
        nc.sync.dma_start(iota_sb, iota_d[:, :])
        onesr_sb = consts.tile([1, 128], f32)
        nc.sync.dma_start(onesr_sb, onesr_d[:, :])
        onesc_sb = consts.tile([128, 1], f32)
        nc.sync.dma_start(onesc_sb, onesc_d[:, :])
        kpos_sb = consts.tile([128, 2], f32)
        nc.sync.dma_start(kpos_sb, kpos_d[:, :])
        kidx_sb = consts.tile([128, 2], f32)
        nc.sync.dma_start(kidx_sb, kidx_d[:, :])
        segrow_sb = consts.tile([1, BPC * HW], f32)
        nc.sync.dma_start(segrow_sb, segrow_d[:, :])
        k255_sb = consts.tile([128, 2], f32)
        nc.sync.dma_start(k255_sb, k255_d[:, :])
        segn_sb = consts.tile([HWC, BPC * NHWC], f32)
        nc.sync.dma_start(segn_sb, segn_d[:, :])
        labown_sb = consts.tile([1, BPC * HW], f32)
        nc.sync.dma_start(labown_sb, labown_d[:, :])
        laball_sb = consts.tile([128, B * HW // 128], f32)
        nc.sync.dma_start(laball_sb, laball_d[:, :])

        # ---- global label sum -> eps*g per-partition scalar -----------
        labsum = small.tile([128, 1], f32, tag="sm")
        nc.vector.reduce_sum(out=labsum, in_=laball_sb, axis=mybir.AxisListType.X)
        g_ps = ps_q.tile([1, 1], f32, tag="qp")
        nc.tensor.matmul(g_ps, labsum, onesc_sb, start=True, stop=True)
        g_sb = small.tile([1, 1], f32, tag="sm")
        nc.scalar.copy(g_sb, g_ps)
        gbc_ps = ps_q.tile([128, 1], f32, tag="qp")
        nc.tensor.matmul(gbc_ps, onesr_sb, g_sb, start=True, stop=True)
        epsg_sb = keep_p.tile([128, 1], f32)
        nc.scalar.activation(out=epsg_sb, in_=gbc_ps, func=Act.Copy,
                             bias=0.0, scale=EPS)

        # ---- persistent accumulators [128, cols] ----------------------
        sproto_t = keep_p.tile([128, 2 * BPC], f32)
        numt = keep_p.tile([128, 2 * BPC], f32)
        nq2t = keep_p.tile([128, 2 * BPC], f32)
        countst = keep_p.tile([128, 2 * BPC], f32)
        nspt = keep_p.tile([128, BPC], f32)

        HH = HW // 2  # 392

        for b in range(BPC):
            # ---- Phase A: s_proto ------------------------------------
            sfc = feat.tile([128, 2, HW], f32, tag="sfc")
            nc.sync.dma_start(sfc, sf_d[b])
            lbc = ps_lab.tile([128, 2, 512], f32, tag="lab", name=f"lab_{b}")
            for h in range(2):
                nc.tensor.matmul(lbc[:, h, 0:HH], onesr_sb,
                                 labown_sb[0:1, b * HW + h * HH:b * HW + (h + 1) * HH],
                                 start=True, stop=True)
            sp4 = small.tile([128, 4], f32, tag="sp4")
            scr = scr_p.tile([128, 2, 2, HH], f32, tag="scr")
            sfcv = sfc.rearrange("p c (h x) -> p c h x", h=2)
            lbcv = (lbc[:, :, 0:HH].unsqueeze(1)
                    .broadcast_to([128, 2, 2, HH]))
            nc.vector.tensor_tensor(out=scr, in0=sfcv, in1=lbcv, op=Alu.mult)
            nc.vector.tensor_reduce(out=sp4, in_=scr,
                                    axis=mybir.AxisListType.X, op=Alu.add)
            sp4v = sp4.rearrange("p (c h) -> p c h", h=2)
            nc.vector.tensor_tensor(out=sproto_t[:, 2 * b:2 * b + 2],
                                    in0=sp4v[:, :, 0], in1=sp4v[:, :, 1],
                                    op=Alu.add)
            scr2s = small.tile([128, 2], f32, tag="sm2")
            nc.scalar.activation(out=scr2s, in_=sproto_t[:, 2 * b:2 * b + 2],
                                 func=Act.Square,
                                 accum_out=nspt[:, b:b + 1])

            # ---- Phase B: transpose qf to [hw, c] chunks -------------
            qfc = feat.tile([128, 2, HW], f32, tag="qfc")
            nc.sync.dma_start(qfc, qf_d[b])
            qfT = []
            for hc in range(NHWC):
                t = qft_p.tile([HWC, 258], f32, tag="qft")
                qfT.append(t)
                nc.vector.memset(t[:, 256:257], 1.0)
                for cc in range(2):
                    pst = ps_t.tile([HWC, 128], f32, tag="pst")
                    nc.tensor.transpose(
                        pst, qfc[:, cc, hc * HWC:(hc + 1) * HWC], id_sb)
                    if cc == 0:
                        nc.vector.tensor_copy(t[:, 0:128], pst)
                    else:
                        nc.scalar.copy(t[:, 128:256], pst)

            # ---- Phase C: onehot [hw, k] via 2 ACT passes ------------
            oh = []
            for hc in range(NHWC):
                o = oh_p.tile([HWC, K], f32, tag="oh")
                oh.append(o)
                nc.scalar.activation(
                    out=o, in_=iota_sb[:HWC, :], func=Act.Abs,
                    bias=segn_sb[:, b * NHWC + hc:b * NHWC + hc + 1], scale=1.0)
                nc.scalar.activation(out=o, in_=o, func=Act.Relu,
                                     bias=1.0, scale=-1.0)

            # ---- Phase D: q_fsum + counts ----------------------------
            qsb = feat.tile([128, 2, 257], f32, tag="qsb")
            for kc in range(2):
                qps = ps_q.tile([128, 257], f32, tag="qp")
                for hc in range(NHWC):
                    nc.tensor.matmul(qps, oh[hc][:, kc * 128:(kc + 1) * 128],
                                     qfT[hc][:, 0:257],
                                     start=(hc == 0), stop=(hc == NHWC - 1))
                nc.scalar.copy(qsb[:, kc, :], qps)
            nc.vector.tensor_copy(countst[:, 2 * b:2 * b + 2], qsb[:, :, 256])

            # ---- Phase E: num / nq2 ----------------------------------
            spt_ps = ps_q.tile([2, 128], f32, tag="qp")
            nc.tensor.transpose(spt_ps, sproto_t[:, 2 * b:2 * b + 2], id_sb)
            spt_sb = small.tile([2, 128], f32, tag="spt")
            nc.scalar.copy(spt_sb, spt_ps)
            sprow = small.tile([1, 256], f32, tag="sprow")
            nc.gpsimd.dma_start(sprow, spt_sb)
            sprep = ps_q.tile([128, 256], f32, tag="qp")
            nc.tensor.matmul(sprep, onesr_sb, sprow, start=True, stop=True)
            scrn = scr_p.tile([128, 2, 256], f32, tag="scrn")
            nc.vector.tensor_tensor(
                out=scrn, in0=qsb[:, :, 0:256],
                in1=sprep.unsqueeze(1).broadcast_to([128, 2, 256]),
                op=Alu.mult)
            nc.vector.tensor_reduce(out=numt[:, 2 * b:2 * b + 2], in_=scrn,
                                    axis=mybir.AxisListType.X, op=Alu.add)
            for kc in range(2):
                scrn2 = scr_p.tile([128, 256], f32, tag="scrn2",
                                   name=f"scrn2_{b}_{kc}")
                nc.scalar.activation(
                    out=scrn2, in_=qsb[:, kc, 0:256], func=Act.Square,
                    accum_out=nq2t[:, 2 * b + kc:2 * b + kc + 1])

        # ---- Phase F: cosine + keep decisions (batched) ---------------
        NB = 2 * BPC
        ns2_ps = ps_q.tile([1, BPC], f32, tag="qp")
        nc.tensor.matmul(ns2_ps, onesc_sb, nspt, start=True, stop=True)
        nsr = small.tile([1, BPC], f32, tag="sm")
        nc.scalar.activation(out=nsr, in_=ns2_ps, func=Act.Sqrt,
                             bias=0.0, scale=1.0)

        pres = keep_p.tile([128, NB], f32)
        nc.vector.tensor_scalar(out=pres, in0=countst, scalar1=0.0,
                                scalar2=None, op0=Alu.is_gt)
        pres3 = pres.rearrange("p (b k) -> p b k", k=2)
        kposb = kpos_sb.unsqueeze(1).broadcast_to([128, BPC, 2])
        nc.vector.tensor_tensor(out=pres3, in0=pres3, in1=kposb, op=Alu.mult)

        npr_ps = ps_q.tile([1, NB], f32, tag="qp")
        nc.tensor.matmul(npr_ps, onesc_sb, pres, start=True, stop=True)
        npr = small.tile([1, NB], f32, tag="sm")
        nc.scalar.copy(npr, npr_ps)
        nprv = npr.rearrange("p (b k) -> p b k", k=2)
        npb = small.tile([1, BPC], f32, tag="sm")
        nc.vector.tensor_tensor(out=npb, in0=nprv[:, :, 0], in1=nprv[:, :, 1],
                                op=Alu.add)
        # bc row: cols 0:BPC = ns, BPC:2*BPC = (npres == 1)
        bcrow = small.tile([1, 2 * BPC], f32, tag="sm")
        nc.vector.tensor_copy(bcrow[:, 0:BPC], nsr)
        nc.vector.tensor_scalar(out=bcrow[:, BPC:2 * BPC], in0=npb,
                                scalar1=1.0, scalar2=None, op0=Alu.is_equal)
        bc_ps = ps_q.tile([128, 2 * BPC], f32, tag="qp")
        nc.tensor.matmul(bc_ps, onesr_sb, bcrow, start=True, stop=True)
        bc_sb = keep_p.tile([128, 2 * BPC], f32)
        nc.scalar.copy(bc_sb, bc_ps)

        sqnq = keep_p.tile([128, NB], f32)
        nc.scalar.activation(out=sqnq, in_=nq2t, func=Act.Sqrt,
                             bias=0.0, scale=1.0)
        den = keep_p.tile([128, NB], f32)
        den3 = den.rearrange("p (b k) -> p b k", k=2)
        nsb3 = bc_sb[:, 0:BPC].unsqueeze(2).broadcast_to([128, BPC, 2])
        nc.vector.tensor_tensor(out=den3, in0=sqnq.rearrange(
            "p (b k) -> p b k", k=2), in1=nsb3, op=Alu.mult)
        ec = keep_p.tile([128, NB], f32)
        cnt1 = keep_p.tile([128, NB], f32)
        nc.vector.tensor_scalar(out=cnt1, in0=countst, scalar1=1.0,
                                scalar2=None, op0=Alu.max)
        nc.vector.tensor_tensor(out=ec, in0=cnt1,
                                in1=epsg_sb.broadcast_to([128, NB]),
                                op=Alu.mult)
        nc.vector.tensor_tensor(out=den, in0=den, in1=ec, op=Alu.max)
        nc.vector.reciprocal(out=den, in_=den)
        cosv = keep_p.tile([128, NB], f32)
        nc.vector.tensor_tensor(out=cosv, in0=numt, in1=den, op=Alu.mult)

        keepv = keep_p.tile([128, NB], f32)
        nc.vector.tensor_scalar(out=keepv, in0=cosv, scalar1=0.0,
                                scalar2=None, op0=Alu.is_ge)
        keep3 = keepv.rearrange("p (b k) -> p b k", k=2)
        k255b = k255_sb.unsqueeze(1).broadcast_to([128, BPC, 2])
        nc.vector.tensor_tensor(out=keep3, in0=keep3, in1=k255b, op=Alu.max)
        nonly3 = bc_sb[:, BPC:2 * BPC].unsqueeze(2).broadcast_to([128, BPC, 2])
        nc.vector.tensor_tensor(out=keep3, in0=keep3, in1=nonly3, op=Alu.max)
        nc.vector.tensor_tensor(out=keepv, in0=keepv, in1=pres, op=Alu.mult)

        # ---- cos output ----------------------------------------------
        cosT_ps = ps_q.tile([NB, 128], f32, tag="qp")
        nc.tensor.transpose(cosT_ps, cosv, id_sb)
        cosT_sb = small.tile([NB, 128], f32, tag="cosT")
        nc.scalar.copy(cosT_sb, cosT_ps)
        nc.sync.dma_start(cos_d[:, :], cosT_sb)

        # ---- final mask: onehotT[k,hw] @ keep column -> 28x28, upsample
        for t in range(2):
            smallt = g_p.tile([4 * HS, HS], f32, tag="smallt",
                              name=f"smallt{t}")
            for bq in range(4):
                b = 4 * t + bq
                segbc = [ps_lab.tile([128, HH], f32, tag="lab",
                                     name=f"segbc_{b}_{h}") for h in range(2)]
                for h in range(2):
                    nc.tensor.matmul(segbc[h], onesr_sb,
                                     segrow_sb[0:1,
                                               b * HW + h * HH:
                                               b * HW + (h + 1) * HH],
                                     start=True, stop=True)
                ohT = feat.tile([128, 2, HW], f32, tag="ohT", name=f"ohT{b}")
                for kc in range(2):
                    for h in range(2):
                        nc.vector.tensor_scalar(
                            out=ohT[:, kc, h * HH:(h + 1) * HH],
                            in0=segbc[h], scalar1=kidx_sb[:, kc:kc + 1],
                            scalar2=None, op0=Alu.is_equal)
                smb_ps = ps_q.tile([HWC, 8], f32, tag="qp", name=f"smb{b}")
                for hc in range(NHWC):
                    for kc in range(2):
                        nc.tensor.matmul(
                            smb_ps[:, hc:hc + 1],
                            ohT[:, kc, hc * HWC:(hc + 1) * HWC],
                            keepv[:, 2 * b + kc:2 * b + kc + 1],
                            start=(kc == 0), stop=(kc == 1))
                smb_sb = small.tile([HWC, 8], f32, tag="smb", name=f"smb_s{b}")
                nc.scalar.activation(out=smb_sb[:, 0:NHWC],
                                     in_=smb_ps[:, 0:NHWC],
                                     func=Act.Copy, bias=0.0, scale=255.0)
                smT_ps = ps_t.tile([NHWC, HWC], f32, tag="pst",
                                   name=f"smT{b}")
                nc.tensor.transpose(smT_ps, smb_sb[:, 0:NHWC],
                                    id_sb[:HWC, :HWC])
                smT_sb = small.tile([NHWC, HWC], f32, tag="smT",
                                    name=f"smT_s{b}")
                nc.vector.tensor_copy(smT_sb, smT_ps)
                # [7 x 112] rows -> 28 (hs) partitions of 28 (ws)
                nc.gpsimd.dma_start(
                    smallt[28 * bq:28 * (bq + 1), :],
                    smT_sb.rearrange("p (s w) -> p s w", w=HS))
            xet = g_p.tile([4 * HS, HQ], f32, tag="xet", name=f"xet{t}")
            nc.vector.tensor_copy(
                xet.rearrange("p (w j) -> p w j", j=UP),
                smallt.unsqueeze(2).broadcast_to([4 * HS, HS, UP]))
            nc.sync.dma_start(
                outm_d[4 * t:4 * t + 4].rearrange(
                    "b (hs j) x -> (b hs) j x", j=UP),
                xet.unsqueeze(1).broadcast_to([4 * HS, UP, HQ]))


    import os
    if os.environ.get("KM_NO_SPLIT") != "1":
        _split_heavy_waits(nc, maxw=1)
    return nc


def _split_heavy_waits(nc, maxw=2):
    """Walrus codegen rejects instructions with >2 sync waits. Move excess
    waits onto same-engine InstNoOp instructions inserted just before."""
    import concourse.mybir as mybir

    counter = [0]
    for f in nc.m.functions:
        for blk in f.blocks:
            insts = list(blk.instructions)
            out = []
            changed = False
            for ins in insts:
                si = ins.sync_info
                waits = list(si.on_wait) if si is not None and si.on_wait else []
                if len(waits) > maxw:
                    extra, keep = waits[:-maxw], waits[-maxw:]
                    for i in range(0, len(extra), maxw):
                        counter[0] += 1
                        nop = mybir.InstNoOp(name=f"WNOP-{counter[0]}")
                        nop.engine = ins.engine
                        nop.sync_info = type(si)(
                            on_wait=extra[i:i + maxw], on_update=[])
                        out.append(nop)
                    si.on_wait = keep
                    changed = True
                out.append(ins)
            if changed:
                blk.instructions = out


def _host_prep(s_feature, small_s_label, q_feature, small_q_mask):
    """Build the per-core input maps (host-side sharding + constants)."""
    sf = np.ascontiguousarray(s_feature, dtype=np.float32).reshape(
        NCORES, BPC, 2, 128, HW)
    qf = np.ascontiguousarray(q_feature, dtype=np.float32).reshape(
        NCORES, BPC, 2, 128, HW)
    lab = np.ascontiguousarray(small_s_label, dtype=np.float32).reshape(B, HW)
    laball = lab.reshape(128, B * HW // 128)
    sqm = np.ascontiguousarray(small_q_mask, dtype=np.int32).reshape(B, HW)

    iota_row = np.tile(np.arange(K, dtype=np.float32), (128, 1))
    ident = np.eye(128, dtype=np.float32)
    ones_row = np.ones((1, 128), np.float32)
    ones_col = np.ones((128, 1), np.float32)
    kvals = np.arange(K).reshape(2, 128).T       # [128, 2]: k = kc*128 + p
    kpos = (kvals > 0).astype(np.float32)
    k255 = (kvals == 255).astype(np.float32)
    kidx = kvals.astype(np.float32)

    in_maps = []
    for c in range(NCORES):
        seg_c = sqm[c * BPC:(c + 1) * BPC]                       # [8, 784]
        segn = -seg_c.reshape(BPC, NHWC, HWC).transpose(2, 0, 1).reshape(
            HWC, BPC * NHWC).astype(np.float32)
        in_maps.append(dict(
            sf=np.ascontiguousarray(sf[c]),
            qf=np.ascontiguousarray(qf[c]),
            label_all=np.ascontiguousarray(laball),
            label_own=np.ascontiguousarray(lab[c * BPC:(c + 1) * BPC].reshape(1, BPC * HW)),
            segn=np.ascontiguousarray(segn),
            iota_row=iota_row,
            ident=ident,
            ones_row=ones_row,
            ones_col=ones_col,
            kpos=np.ascontiguousarray(kpos),
            k255=np.ascontiguousarray(k255),
            kidx=np.ascontiguousarray(kidx),
            segrow=np.ascontiguousarray(
                seg_c.reshape(1, BPC * HW).astype(np.float32)),
        ))
    return in_maps


def _assemble(results):
    final = np.empty((B, 1, HQ, HQ), np.float32)
    cos = np.empty((B, K), np.float32)
    for c in range(NCORES):
        final[c * BPC:(c + 1) * BPC, 0] = results[c]["outm"]
        cos[c * BPC:(c + 1) * BPC] = results[c]["cos_out"].reshape(BPC, K)
    return final, cos


def kernel(s_feature, small_s_label, q_feature, small_q_mask, query_mask):
    from concourse.bass_utils import run_bass_kernel_spmd

    if "nc" not in _CACHE:
        _CACHE["nc"] = _build_nc()
    nc = _CACHE["nc"]

    in_maps = _host_prep(s_feature, small_s_label, q_feature, small_q_mask)
    res = run_bass_kernel_spmd(nc, in_maps, core_ids=list(range(NCORES)))
    kernel._last_results = res
    final, cos = _assemble(res.results)

    # Safety net: the device path assumes query_mask == repeat16(small_q_mask)
    # (always true for reference.setup_inputs()). If violated, redo the final
    # gather on the host from the device-computed cos.
    sqm = np.asarray(small_q_mask)
    qm = np.asarray(query_mask)
    up = np.repeat(np.repeat(sqm, UP, axis=2), UP, axis=3)
    if not np.array_equal(up, qm):
        lab_sum = np.float32(np.asarray(small_s_label, dtype=np.float32).sum())
        seg = sqm[:, 0].reshape(B, HW)
        counts = np.zeros((B, K), np.float32)
        for bi in range(B):
            np.add.at(counts[bi], seg[bi], 1.0)
        v = np.arange(K)
        present = (counts > 0) & (v[None, :] > 0)
        npres = present.sum(axis=1, keepdims=True)
        keepf = (present & ((cos >= 0) | (v[None, :] == 255) | (npres == 1))
                 ).astype(np.float32) * 255.0
        idx = qm[:, 0].reshape(B, -1)
        final = np.take_along_axis(keepf, idx, axis=1).reshape(B, 1, HQ, HQ)

    return final.astype(np.float32), cos.astype(np.float32)
